# revision 13
# baseline (speedup 1.0000x reference)
"""Trainium2 Bass kernel for a tanh RNN cell (CustomRNNCell).

Reference computation:
    xp = einsum("bti,hi->bth", x_seq, W_x) + b_x          # input projection
    h_{t+1} = tanh(h_t @ W_h.T + b_h + xp[:, t])           # 512 sequential steps
    return h_final                                          # [B, H]

Shapes: B=64, T=512, INPUT_DIM=HIDDEN_DIM=1024, fp32 in/out.

Strategy (data-parallel over batch, 8 cores x 8 batch rows each):
  - Everything on-chip per core. Host pre-transposes x/W into lhsT-friendly
    fp16 layouts (fp16 keeps FWL weight loads at 2 cols/cycle and end-to-end
    error ~4e-4 measured against the fp32 reference in numpy).
  - Phase 1 (xp GEMM): out.T-layout GEMM, W_x stationary. psum[m] (128 H
    rows) x 512 timesteps per batch row; ScalarE epilogue adds (b_x + b_h)
    as a per-partition bias and stores fp16 xp into SBUF in a
    [128, T, (m*8+b)] layout so each recurrence step reads one contiguous
    [128, 64] slice.
  - Phase 2 (recurrence): h kept resident in SBUF in transposed layout
    h.T = [H-chunk on partitions, batch in free dim], so no per-step
    transposes. Each step: one fp16 identity matmul injects c_t = xp_t into
    PSUM (start=True; DVE preload would be clobbered by has_written
    semantics), then 64 accumulating [128x128]@[128x8] matmuls apply W_h
    (W-stationary; LDWEIGHTS-bound at ~53ns/tile with FWL), then one
    ScalarE tanh produces the next h (fp16). Hardware For_i loop, 16 steps
    per body, to keep the instruction stream small.

Output is returned per-core as h.T [128, 64] fp32; the host untransposes.
"""

import os

import numpy as np

import concourse.bass as bass
import concourse.mybir as mybir
import concourse.tile as tile
from concourse import bacc
from concourse.bass import ds, ts
from concourse.bass_utils import run_bass_kernel_spmd

B, T, I, H = 64, 512, 1024, 1024
NCORES = 8
BL = B // NCORES           # batch rows per core
KC = I // 128              # contraction chunks (8)
MC = H // 128              # output H chunks (8)
MB = MC * BL               # 64 columns in recurrence psum layout
UNROLL = 32

F16 = mybir.dt.float16
F32 = mybir.dt.float32
AF = mybir.ActivationFunctionType


def build_bass(t_steps=T):
    nc = bacc.Bacc(None)

    x_t = nc.dram_tensor("xT", [I, BL * T], F16, kind="ExternalInput")
    wx_t = nc.dram_tensor("WxT", [I, H], F16, kind="ExternalInput")
    wh_t = nc.dram_tensor("WhT", [H, H], F16, kind="ExternalInput")
    bias = nc.dram_tensor("bias", [128, MC], F32, kind="ExternalInput")
    ident = nc.dram_tensor("ident", [128, 128], F16, kind="ExternalInput")
    h_out = nc.dram_tensor("hT_out", [128, MB], F32, kind="ExternalOutput")

    with tile.TileContext(nc) as tc:
        with (
            tc.tile_pool(name="const", bufs=1) as const_pool,
            tc.tile_pool(name="state", bufs=1) as state_pool,
            tc.tile_pool(name="xtiles", bufs=2) as x_pool,
            tc.tile_pool(name="ps_gemm", bufs=4, space="PSUM") as psg_pool,
            tc.tile_pool(name="ps_rec", bufs=1, space="PSUM") as psr_pool,
        ):
            wx_sb = const_pool.tile([128, KC, H], F16)
            nc.sync.dma_start(wx_sb[:], wx_t.rearrange("(ko ki) h -> ki ko h", ki=128))
            wh_sb = const_pool.tile([128, KC, H], F16)
            nc.sync.dma_start(wh_sb[:], wh_t.rearrange("(ko ki) h -> ki ko h", ki=128))
            bias_sb = const_pool.tile([128, MC], F32)
            nc.sync.dma_start(bias_sb[:], bias[:])
            id_sb = const_pool.tile([128, 128], F16)
            nc.sync.dma_start(id_sb[:], ident[:])

            # xp in [p, (t, m*8+b)] layout, fp16 (flat so the recurrence can
            # take a register-dynamic contiguous slice; 3D view for the
            # static strided epilogue writes)
            xp_sb = state_pool.tile([128, T * MB], F16)
            xp3 = xp_sb.rearrange("p (t m) -> p t m", m=MB)
            # h.T state, ping-pong in dim 1: h_sb[p, par, m*8+b]
            h_sb = state_pool.tile([128, 2, MB], F16)
            nc.any.memzero(h_sb[:])

            xt_view = x_t.rearrange("(ko ki) tok -> ki ko tok", ki=128)

            # ---- Phase 1: xp = x @ W_x.T (+ bias) in out.T layout ----
            for b in range(BL):
                xt_b = x_pool.tile([128, KC, T], F16)
                nc.sync.dma_start(xt_b[:], xt_view[:, :, b * T : (b + 1) * T])
                for m in range(MC):
                    ps = psg_pool.tile([128, T], F32)
                    for k in range(KC):
                        nc.tensor.matmul(
                            ps[:],
                            wx_sb[:, k, m * 128 : (m + 1) * 128],
                            xt_b[:, k, :],
                            start=(k == 0),
                            stop=(k == KC - 1),
                        )
                    # xp[:, :, m*8+b] = psum + bias[m]  (per-partition bias)
                    nc.scalar.activation(
                        xp3[:, :, m * BL + b],
                        ps[:],
                        AF.Identity,
                        bias=bias_sb[:, m : m + 1],
                        scale=1.0,
                    )

            # ---- Phase 2: recurrence ----
            ps_rec0 = psr_pool.tile([128, MB], F32, tag="ps_rec0")
            ps_rec1 = psr_pool.tile([128, MB], F32, tag="ps_rec1")
            ps_t = [ps_rec0, ps_rec1]

            with tc.For_i(
                0, t_steps, UNROLL, hint_engines=(mybir.EngineType.PE,)
            ) as iv:
                for u in range(UNROLL):
                    ps = ps_t[u % 2]
                    cur = h_sb[:, u % 2, :]
                    nxt = h_sb[:, (u + 1) % 2, :]
                    # psum <- c_t (identity matmul sets has_written)
                    nc.tensor.matmul(
                        ps[:],
                        id_sb[:],
                        xp_sb[:, ts(iv + u, MB)],
                        start=True,
                        stop=False,
                    )
                    for m in range(MC):
                        for k in range(KC):
                            nc.tensor.matmul(
                                ps[:, m * BL : (m + 1) * BL],
                                wh_sb[:, k, m * 128 : (m + 1) * 128],
                                cur[:, k * BL : (k + 1) * BL],
                                start=False,
                                stop=(m == MC - 1 and k == KC - 1),
                            )
                    nc.scalar.activation(nxt, ps[:], AF.Tanh)

            # Final h in fp32 straight from the last psum (t = T-1 used
            # ps_t[(UNROLL-1) % 2]).
            out_sb = state_pool.tile([128, MB], F32)
            nc.scalar.activation(out_sb[:], ps_t[(UNROLL - 1) % 2][:], AF.Tanh)
            nc.sync.dma_start(h_out[:], out_sb[:])

    nc.finalize()  # Bacc: run reg-alloc + wait-splitting passes
    return nc


_NC_CACHE = None


def _get_nc():
    global _NC_CACHE
    if _NC_CACHE is None:
        _NC_CACHE = build_bass()
    return _NC_CACHE


def kernel(x_seq, W_h, b_h, W_x, b_x, _want_results=False, **run_kwargs):
    x_seq = np.asarray(x_seq, dtype=np.float32)
    W_h = np.asarray(W_h, dtype=np.float32)
    b_h = np.asarray(b_h, dtype=np.float32)
    W_x = np.asarray(W_x, dtype=np.float32)
    b_x = np.asarray(b_x, dtype=np.float32)

    wx_t = np.ascontiguousarray(W_x.T).astype(np.float16)
    wh_t = np.ascontiguousarray(W_h.T).astype(np.float16)
    bias = np.ascontiguousarray(
        (b_x + b_h).astype(np.float32).reshape(MC, 128).T
    )
    ident = np.eye(128, dtype=np.float16)

    in_maps = []
    for c in range(NCORES):
        x2d = x_seq[c * BL : (c + 1) * BL].reshape(BL * T, I)
        x_t = np.ascontiguousarray(x2d.T.astype(np.float16))
        in_maps.append(
            {"xT": x_t, "WxT": wx_t, "WhT": wh_t, "bias": bias, "ident": ident}
        )

    nc = _get_nc()
    res = run_bass_kernel_spmd(nc, in_maps, core_ids=list(range(NCORES)), **run_kwargs)

    out = np.empty((B, H), dtype=np.float32)
    for c in range(NCORES):
        h_t = res.results[c]["hT_out"]  # [128, MB]; h_t[p, m*8+b]
        out[c * BL : (c + 1) * BL] = (
            h_t.reshape(128, MC, BL).transpose(2, 1, 0).reshape(BL, H)
        )
    if _want_results:
        return out, res
    return out


if __name__ == "__main__":
    rng = np.random.default_rng(0)
    inputs = {
        "x_seq": rng.standard_normal((B, T, I)).astype(np.float32),
        "W_h": rng.uniform(-1 / 32, 1 / 32, (H, H)).astype(np.float32),
        "b_h": rng.uniform(-1 / 32, 1 / 32, H).astype(np.float32),
        "W_x": rng.uniform(-1 / 32, 1 / 32, (H, I)).astype(np.float32),
        "b_x": rng.uniform(-1 / 32, 1 / 32, I).astype(np.float32),
    }
    out = kernel(**inputs)
    print("kernel output", out.shape, out.dtype, np.abs(out).max())


# revision 14
# speedup vs baseline: 33.8364x; 33.8364x over previous
"""Trainium2 Bass kernel for a tanh RNN cell (CustomRNNCell).

Reference computation:
    xp = einsum("bti,hi->bth", x_seq, W_x) + b_x          # input projection
    h_{t+1} = tanh(h_t @ W_h.T + b_h + xp[:, t])           # 512 sequential steps
    return h_final                                          # [B, H]

Shapes: B=64, T=512, INPUT_DIM=HIDDEN_DIM=1024, fp32 in/out.

Strategy (data-parallel over batch, 8 cores x 8 batch rows each):
  - Everything on-chip per core. Host pre-transposes x/W into lhsT-friendly
    fp16 layouts (fp16 keeps FWL weight loads at 2 cols/cycle and end-to-end
    error ~4e-4 measured against the fp32 reference in numpy).
  - Phase 1 (xp GEMM): out.T-layout GEMM, W_x stationary. psum[m] (128 H
    rows) x 512 timesteps per batch row; ScalarE epilogue adds (b_x + b_h)
    as a per-partition bias and stores fp16 xp into SBUF in a
    [128, T, (m*8+b)] layout so each recurrence step reads one contiguous
    [128, 64] slice.
  - Phase 2 (recurrence): h kept resident in SBUF in transposed layout
    h.T = [H-chunk on partitions, batch in free dim], so no per-step
    transposes. Each step: one fp16 identity matmul injects c_t = xp_t into
    PSUM (start=True; DVE preload would be clobbered by has_written
    semantics), then 64 accumulating [128x128]@[128x8] matmuls apply W_h
    (W-stationary; LDWEIGHTS-bound at ~53ns/tile with FWL), then one
    ScalarE tanh produces the next h (fp16). Hardware For_i loop, 16 steps
    per body, to keep the instruction stream small.

Output is returned per-core as h.T [128, 64] fp32; the host untransposes.
"""

import os

import numpy as np

import concourse.bass as bass
import concourse.mybir as mybir
import concourse.tile as tile
from concourse import bacc
from concourse.bass import ds, ts
from concourse.bass_utils import run_bass_kernel_spmd

B, T, I, H = 64, 512, 1024, 1024
NCORES = 8
BL = B // NCORES           # batch rows per core
KC = I // 128              # contraction chunks (8)
MC = H // 128              # output H chunks (8)
MB = MC * BL               # 64 columns in recurrence psum layout
UNROLL = 32

F16 = mybir.dt.float16
F32 = mybir.dt.float32
AF = mybir.ActivationFunctionType


def build_bass(t_steps=T):
    nc = bacc.Bacc(None)

    x_t = nc.dram_tensor("xT", [I, BL * T], F16, kind="ExternalInput")
    wx_t = nc.dram_tensor("WxT", [I, H], F16, kind="ExternalInput")
    wh_t = nc.dram_tensor("WhT", [H, H], F16, kind="ExternalInput")
    bias = nc.dram_tensor("bias", [128, MC], F32, kind="ExternalInput")
    ident = nc.dram_tensor("ident", [128, 128], F16, kind="ExternalInput")
    h_out = nc.dram_tensor("hT_out", [128, MB], F32, kind="ExternalOutput")

    with tile.TileContext(nc) as tc:
        with (
            tc.tile_pool(name="const", bufs=1) as const_pool,
            tc.tile_pool(name="state", bufs=1) as state_pool,
            tc.tile_pool(name="xtiles", bufs=2) as x_pool,
            tc.tile_pool(name="ps_gemm", bufs=4, space="PSUM") as psg_pool,
            tc.tile_pool(name="ps_rec", bufs=1, space="PSUM") as psr_pool,
        ):
            wx_sb = const_pool.tile([128, KC, H], F16)
            nc.sync.dma_start(wx_sb[:], wx_t.rearrange("(ko ki) h -> ki ko h", ki=128))
            wh_sb = const_pool.tile([128, KC, H], F16)
            nc.sync.dma_start(wh_sb[:], wh_t.rearrange("(ko ki) h -> ki ko h", ki=128))
            bias_sb = const_pool.tile([128, MC], F32)
            nc.sync.dma_start(bias_sb[:], bias[:])
            id_sb = const_pool.tile([128, 128], F16)
            nc.sync.dma_start(id_sb[:], ident[:])

            # xp in [p, (t, m*8+b)] layout, fp16 (flat so the recurrence can
            # take a register-dynamic contiguous slice; 3D view for the
            # static strided epilogue writes)
            xp_sb = state_pool.tile([128, T * MB], F16)
            xp3 = xp_sb.rearrange("p (t m) -> p t m", m=MB)
            # h.T state, ping-pong in dim 1: h_sb[p, par, m*8+b]
            h_sb = state_pool.tile([128, 2, MB], F16)
            nc.any.memzero(h_sb[:])

            xt_view = x_t.rearrange("(ko ki) tok -> ki ko tok", ki=128)

            # ---- Phase 1: xp = x @ W_x.T (+ bias) in out.T layout ----
            for b in range(BL):
                xt_b = x_pool.tile([128, KC, T], F16)
                nc.sync.dma_start(xt_b[:], xt_view[:, :, b * T : (b + 1) * T])
                for m in range(MC):
                    ps = psg_pool.tile([128, T], F32)
                    for k in range(KC):
                        nc.tensor.matmul(
                            ps[:],
                            wx_sb[:, k, m * 128 : (m + 1) * 128],
                            xt_b[:, k, :],
                            start=(k == 0),
                            stop=(k == KC - 1),
                        )
                    # xp[:, :, m*8+b] = psum + bias[m]  (per-partition bias)
                    nc.scalar.activation(
                        xp3[:, :, m * BL + b],
                        ps[:],
                        AF.Identity,
                        bias=bias_sb[:, m : m + 1],
                        scale=1.0,
                    )

            # ---- Phase 2: recurrence ----
            # Two PSUM banks per step (lo = m-chunks 0..3, hi = 4..7) so the
            # lo-half tanh overlaps the hi-half matmuls (different banks —
            # same-bank PE-write + ACT-read would be fatal), and the next
            # step's early matmuls only wait on the half they read.
            HB = MB // 2  # 32 cols per half

            ps_lo = [
                psr_pool.tile([128, HB], F32, tag="ps_lo0", name="ps_lo0"),
                psr_pool.tile([128, HB], F32, tag="ps_lo1", name="ps_lo1"),
            ]
            ps_hi = [
                psr_pool.tile([128, HB], F32, tag="ps_hi0", name="ps_hi0"),
                psr_pool.tile([128, HB], F32, tag="ps_hi1", name="ps_hi1"),
            ]

            with tc.For_i(
                0, t_steps, UNROLL, hint_engines=(mybir.EngineType.PE,)
            ) as iv:
                for u in range(UNROLL):
                    plo = ps_lo[u % 2]
                    phi = ps_hi[u % 2]
                    cur = h_sb[:, u % 2, :]
                    nxt = h_sb[:, (u + 1) % 2, :]
                    # psum <- c_t (identity matmuls set has_written)
                    nc.tensor.matmul(
                        plo[:], id_sb[:], xp_sb[:, ds((iv + u) * MB, HB)],
                        start=True, stop=False,
                    )
                    nc.tensor.matmul(
                        phi[:], id_sb[:], xp_sb[:, ds((iv + u) * MB + HB, HB)],
                        start=True, stop=False,
                    )
                    for m in range(MC // 2):
                        for k in range(KC):
                            nc.tensor.matmul(
                                plo[:, m * BL : (m + 1) * BL],
                                wh_sb[:, k, m * 128 : (m + 1) * 128],
                                cur[:, k * BL : (k + 1) * BL],
                                start=False,
                                stop=(m == MC // 2 - 1 and k == KC - 1),
                            )
                    nc.scalar.activation(nxt[:, :HB], plo[:], AF.Tanh)
                    for m in range(MC // 2, MC):
                        for k in range(KC):
                            nc.tensor.matmul(
                                phi[:, (m - MC // 2) * BL : (m - MC // 2 + 1) * BL],
                                wh_sb[:, k, m * 128 : (m + 1) * 128],
                                cur[:, k * BL : (k + 1) * BL],
                                start=False,
                                stop=(m == MC - 1 and k == KC - 1),
                            )
                    nc.scalar.activation(nxt[:, HB:], phi[:], AF.Tanh)

            # Final h in fp32 straight from the last psums.
            out_sb = state_pool.tile([128, MB], F32)
            nc.scalar.activation(out_sb[:, :HB], ps_lo[(UNROLL - 1) % 2][:], AF.Tanh)
            nc.scalar.activation(out_sb[:, HB:], ps_hi[(UNROLL - 1) % 2][:], AF.Tanh)
            nc.sync.dma_start(h_out[:], out_sb[:])

    nc.finalize()  # Bacc: run reg-alloc + wait-splitting passes
    return nc


_NC_CACHE = None


def _get_nc():
    global _NC_CACHE
    if _NC_CACHE is None:
        _NC_CACHE = build_bass()
    return _NC_CACHE


def kernel(x_seq, W_h, b_h, W_x, b_x, _want_results=False, **run_kwargs):
    x_seq = np.asarray(x_seq, dtype=np.float32)
    W_h = np.asarray(W_h, dtype=np.float32)
    b_h = np.asarray(b_h, dtype=np.float32)
    W_x = np.asarray(W_x, dtype=np.float32)
    b_x = np.asarray(b_x, dtype=np.float32)

    wx_t = np.ascontiguousarray(W_x.T).astype(np.float16)
    wh_t = np.ascontiguousarray(W_h.T).astype(np.float16)
    bias = np.ascontiguousarray(
        (b_x + b_h).astype(np.float32).reshape(MC, 128).T
    )
    ident = np.eye(128, dtype=np.float16)

    in_maps = []
    for c in range(NCORES):
        x2d = x_seq[c * BL : (c + 1) * BL].reshape(BL * T, I)
        x_t = np.ascontiguousarray(x2d.T.astype(np.float16))
        in_maps.append(
            {"xT": x_t, "WxT": wx_t, "WhT": wh_t, "bias": bias, "ident": ident}
        )

    nc = _get_nc()
    res = run_bass_kernel_spmd(nc, in_maps, core_ids=list(range(NCORES)), **run_kwargs)

    out = np.empty((B, H), dtype=np.float32)
    for c in range(NCORES):
        h_t = res.results[c]["hT_out"]  # [128, MB]; h_t[p, m*8+b]
        out[c * BL : (c + 1) * BL] = (
            h_t.reshape(128, MC, BL).transpose(2, 1, 0).reshape(BL, H)
        )
    if _want_results:
        return out, res
    return out


if __name__ == "__main__":
    rng = np.random.default_rng(0)
    inputs = {
        "x_seq": rng.standard_normal((B, T, I)).astype(np.float32),
        "W_h": rng.uniform(-1 / 32, 1 / 32, (H, H)).astype(np.float32),
        "b_h": rng.uniform(-1 / 32, 1 / 32, H).astype(np.float32),
        "W_x": rng.uniform(-1 / 32, 1 / 32, (H, I)).astype(np.float32),
        "b_x": rng.uniform(-1 / 32, 1 / 32, I).astype(np.float32),
    }
    out = kernel(**inputs)
    print("kernel output", out.shape, out.dtype, np.abs(out).max())
